# revision 3
# baseline (speedup 1.0000x reference)
"""NoisyTopkRouter Trainium2 kernel.

Math (per batch b, data-parallel over 8 cores):
  h      = gelu(x @ W1 + b1)                         [T, H]
  scores = (h @ W2 + b2) @ TQ.T                      [T, E]
         = h @ (W2 @ TQ.T) + (b2 @ TQ.T)             (second matmul folded)
  gate   = sigmoid(x @ noise_w + noise_b)            [T, 1]
  noisy  = scores + temp * noise * gate
  top-2 -> masked softmax(noisy / (temp + 1e-6))

Device-side layout trick: x is shipped pre-transposed (xT [C, T]) so the
main matmul produces hT [H-slice, tok] tiles directly; the gate is computed
as an extra (33rd) H-slice of the same matmul whose activation is Sigmoid
instead of Gelu, and a one-hot column in the folded second-matmul weights
transposes it back to token-major for free.

f32r matmuls: full PE rate at ~1e-4 relative accuracy (vs 4x slower fp32).
"""

import numpy as np

import concourse.mybir as mybir
import concourse.tile as tile
from concourse import bacc
from concourse.bass_utils import run_bass_kernel_spmd

B, T, C, E, TOPK = 8, 4096, 1024, 8, 2
H = 4 * C
HS_N = H // 128 + 1  # 33 H-slices: 32 real + 1 aug (gate)
HAUG = HS_N * 128  # 4224
KC_N = C // 128  # 8 k-chunks
TT = 512  # tokens per tile
TT_N = T // TT  # 8 token tiles
TS_N = TT // 128  # 4 token slices per tile
SW = 10  # stage-2 width: 8 experts + gate + pad (f32r needs even free dim)

F32 = mybir.dt.float32
F32R = mybir.dt.float32r
U32 = mybir.dt.uint32
AF = mybir.ActivationFunctionType
ALU = mybir.AluOpType

_CACHE = {}


def _build_nc(invtau):
    nc = bacc.Bacc(None, target_bir_lowering=False, debug=False)

    d_xT = nc.dram_tensor("xT", [C, T], F32R, kind="ExternalInput")
    d_w1 = nc.dram_tensor("w1aug", [C, HAUG], F32R, kind="ExternalInput")
    d_w2 = nc.dram_tensor("w2eaug", [HAUG, SW], F32R, kind="ExternalInput")
    d_b1 = nc.dram_tensor("b1aug", [HAUG], F32, kind="ExternalInput")
    d_sc = nc.dram_tensor("scoreconst", [128, E], F32, kind="ExternalInput")
    d_nz = nc.dram_tensor("noisepre", [T, E], F32, kind="ExternalInput")
    d_ro = nc.dram_tensor("router", [T, E], F32, kind="ExternalOutput")
    d_ix = nc.dram_tensor("topk", [T, TOPK], U32, kind="ExternalOutput")

    with tile.TileContext(nc) as tc:
        with (
            tc.tile_pool(name="res", bufs=1) as res,
            tc.tile_pool(name="xp", bufs=2) as xp,
            tc.tile_pool(name="hp", bufs=3) as hp,
            tc.tile_pool(name="np_", bufs=2) as npool,
            tc.tile_pool(name="sp", bufs=3) as sp,
            tc.tile_pool(name="op", bufs=2) as op,
            tc.tile_pool(name="psh", bufs=3, space="PSUM") as psh,
            tc.tile_pool(name="pss", bufs=5, space="PSUM") as pss,
        ):
            # resident weights
            w1t = res.tile([128, KC_N, HAUG], F32R, tag="w1t")
            for kc in range(KC_N):
                nc.sync.dma_start(
                    w1t[:, kc, :], d_w1[kc * 128 : (kc + 1) * 128, :]
                )
            w2t = res.tile([128, HS_N, SW], F32R, tag="w2t")
            nc.sync.dma_start(w2t, d_w2.rearrange("(c p) e -> p c e", p=128))
            b1t = res.tile([128, HS_N], F32, tag="b1t")
            nc.sync.dma_start(b1t, d_b1.rearrange("(c p) -> p c", p=128))
            sct = res.tile([128, E], F32, tag="sct")
            nc.sync.dma_start(sct, d_sc[:])

            for tt in range(TT_N):
                xt = xp.tile([128, KC_N, TT], F32R, tag="xt")
                nc.sync.dma_start(
                    xt,
                    d_xT.rearrange("(c p) t -> p c t", p=128)[
                        :, :, tt * TT : (tt + 1) * TT
                    ],
                )
                noiz = npool.tile([128, TS_N, E], F32, tag="noiz")
                nc.sync.dma_start(
                    noiz,
                    d_nz[tt * TT : (tt + 1) * TT, :].rearrange(
                        "(s p) e -> p s e", p=128
                    ),
                )

                pscore = [
                    pss.tile([128, SW], F32, tag="pss", name=f"pss{i}")
                    for i in range(TS_N)
                ]
                for hs in range(HS_N):
                    ph = psh.tile([128, TT], F32, tag="psh")
                    for kc in range(KC_N):
                        nc.tensor.matmul(
                            ph,
                            w1t[:, kc, hs * 128 : (hs + 1) * 128],
                            xt[:, kc, :],
                            start=(kc == 0),
                            stop=(kc == KC_N - 1),
                        )
                    ht = hp.tile([128, TT], F32R, tag="ht")
                    nc.scalar.activation(
                        ht,
                        ph,
                        AF.Gelu if hs < HS_N - 1 else AF.Sigmoid,
                        bias=b1t[:, hs : hs + 1],
                    )
                    for ts in range(TS_N):
                        nc.tensor.matmul(
                            pscore[ts],
                            ht[:, ts * 128 : (ts + 1) * 128],
                            w2t[:, hs, :],
                            start=(hs == 0),
                            stop=(hs == HS_N - 1),
                        )

                rout = op.tile([128, TS_N, E], F32, tag="rout")
                idxo = op.tile([128, TS_N, TOPK], U32, tag="idxo")
                for ts in range(TS_N):
                    ps = pscore[ts]
                    # noisy = noise_pre * gate + scores + score_const
                    tmp = sp.tile([128, E], F32, tag="tmp")
                    nc.vector.scalar_tensor_tensor(
                        tmp,
                        noiz[:, ts, :],
                        ps[:, 8:9],
                        ps[:, 0:8],
                        op0=ALU.mult,
                        op1=ALU.add,
                    )
                    noisy = sp.tile([128, E], F32, tag="noisy")
                    nc.vector.tensor_add(noisy, tmp, sct)
                    m8 = sp.tile([128, 8], F32, tag="m8")
                    i8 = sp.tile([128, 8], U32, tag="i8")
                    nc.vector.max(out=m8, in_=noisy)
                    nc.vector.max_index(out=i8, in_max=m8, in_values=noisy)
                    nc.vector.tensor_copy(idxo[:, ts, :], i8[:, 0:TOPK])
                    # p2 = sigmoid((v2 - v1) * invtau); p1 = 1 - p2
                    nv1 = sp.tile([128, 1], F32, tag="nv1")
                    nc.vector.tensor_scalar_mul(nv1, m8[:, 0:1], -invtau)
                    p2 = sp.tile([128, 1], F32, tag="p2")
                    nc.scalar.activation(
                        p2, m8[:, 1:2], AF.Sigmoid, bias=nv1, scale=invtau
                    )
                    pd = sp.tile([128, 1], F32, tag="pd")
                    nc.vector.tensor_scalar(
                        pd, p2, -2.0, 1.0, op0=ALU.mult, op1=ALU.add
                    )
                    # out = (noisy>=v2)*p2 + (noisy>=v1)*(p1-p2)
                    a_t = sp.tile([128, E], F32, tag="a_t")
                    nc.vector.tensor_scalar(
                        a_t, noisy, m8[:, 1:2], p2, op0=ALU.is_ge, op1=ALU.mult
                    )
                    b_t = sp.tile([128, E], F32, tag="b_t")
                    nc.vector.tensor_scalar(
                        b_t, noisy, m8[:, 0:1], pd, op0=ALU.is_ge, op1=ALU.mult
                    )
                    nc.vector.tensor_add(rout[:, ts, :], a_t, b_t)

                nc.sync.dma_start(
                    d_ro[tt * TT : (tt + 1) * TT, :].rearrange(
                        "(s p) e -> p s e", p=128
                    ),
                    rout,
                )
                nc.sync.dma_start(
                    d_ix[tt * TT : (tt + 1) * TT, :].rearrange(
                        "(s p) k -> p s k", p=128
                    ),
                    idxo,
                )

    nc.compile()
    return nc


def _prep(x, noise, W1, b1, W2, b2, type_queries, noise_w, noise_b, temperature):
    temp = float(np.asarray(temperature))
    invtau = 1.0 / (temp + 1e-6)

    w1aug = np.zeros((C, HAUG), np.float32)
    w1aug[:, :H] = W1
    w1aug[:, H] = noise_w[:, 0]

    b1aug = np.zeros((HAUG,), np.float32)
    b1aug[:H] = b1
    b1aug[H] = noise_b[0]

    w2eaug = np.zeros((HAUG, SW), np.float32)
    w2eaug[:H, :E] = (W2.astype(np.float64) @ type_queries.astype(np.float64).T).astype(
        np.float32
    )
    w2eaug[H, 8] = 1.0

    score_const = (b2.astype(np.float64) @ type_queries.astype(np.float64).T).astype(
        np.float32
    )
    sct = np.broadcast_to(score_const[None, :], (128, E)).copy()

    noise_pre = (temp * np.asarray(noise, np.float64)).astype(np.float32)

    in_maps = []
    for b in range(B):
        in_maps.append(
            {
                "xT": np.ascontiguousarray(np.asarray(x[b], np.float32).T),
                "w1aug": w1aug,
                "w2eaug": w2eaug,
                "b1aug": b1aug,
                "scoreconst": sct,
                "noisepre": noise_pre[b],
            }
        )
    return invtau, in_maps


def _run(inputs, trace=False):
    invtau, in_maps = _prep(**inputs)
    key = round(invtau, 9)
    if key not in _CACHE:
        _CACHE[key] = _build_nc(invtau)
    nc = _CACHE[key]
    res = run_bass_kernel_spmd(
        nc, in_maps, core_ids=list(range(B)), trace=trace
    )
    router = np.stack([r["router"] for r in res.results]).astype(np.float32)
    topk = np.stack([r["topk"] for r in res.results]).astype(np.int32)
    return (router, topk), res


def kernel(**inputs):
    out, _ = _run(inputs, trace=False)
    return out
